# revision 5
# baseline (speedup 1.0000x reference)
"""Trainium2 Bass kernel for nn_ChunkedKasminaLayer (moe_routing).

Problem (B=8192, D_IN=1024, D_OUT=4096, S=64 seeds, C=64 chunk, H=128):
    base = x @ Wb.T + bb                     [B, D_OUT]
    y    = base.view(B, S, C)
    h    = relu(y @ W1[s] + b1[s])           per seed  [B, S, H]
    bp   = h @ W2[s] + b2[s]                 per seed  [B, S, C]
    out  = (1-a)*y + a*bp,  a = alpha*active (per seed)

Strategy: data-parallel over batch across 8 NeuronCores (1024 rows each,
weights replicated, no collectives). Everything is computed in transposed
layout (features on partitions, batch on the free axis, 512-wide tiles):

  - base:  8 accumulating fp32r matmuls per [128, 512] tile
           (lhsT = Wb.T k-tile, rhs = x.T k-tile), bias via fused copy.
  - seeds: each 128-row group of base.T is a pair of seeds (2 x 64 chunk
           rows). Layer 1 runs the pair as two concurrent K=64 matmuls via
           tile_position (0,0)/(64,0) with lhsT = W1[s] (fp32r); relu+b1
           fused into the PSUM->SBUF copy on the scalar engine (bf16 out).
  - layer2+blend: one PSUM group per pair = diag(1-a) fp32r matmul over
           y.T (full 128 partitions, computes the (1-a)*y term) + two bf16
           matmuls with lhsT = alpha*W2[s] into the two 64-partition
           halves (fp32r cannot write PSUM at offset partitions; bf16 can).
           b2' = alpha*b2 bias fused into the output copy.

fp32r (TF32-like, ~2e-4 matmul error) everywhere except the W2/h path
(bf16, ~2e-3) keeps the end-to-end error at ~2.5e-3 absmax-relative while
running every matmul at 1 column/cycle.
"""

import numpy as np
import ml_dtypes

from concourse import bacc, mybir
from concourse import bass_utils
import concourse.tile as tile

B, D_IN, D_OUT, S = 8192, 1024, 4096, 64
C = D_OUT // S            # 64
H = 2 * C                 # 128
NCORES = 8
BC = B // NCORES          # 1024 batch rows per core
KT = D_IN // 128          # 8 k-tiles
MP = D_OUT // 128         # 32 m-tiles (= seed pairs)
NT = 512                  # batch tile (free dim)
NN = BC // NT             # 2 batch tiles per core

F32 = mybir.dt.float32
F32R = mybir.dt.float32r
BF16 = mybir.dt.bfloat16


def build(reps: int = 1, hw_loop: bool = False):
    """Build + compile the per-core Tile program (same program on all cores).

    reps>1 repeats the whole computation (identical results) — used only for
    wall-clock slope timing. With hw_loop=True the repetition is a tc.For_i
    hardware loop, so the instruction count stays flat.
    """
    nc = bacc.Bacc("TRN2", target_bir_lowering=False, debug=False)

    t_xT = nc.dram_tensor("xT", [128, KT, BC], F32R, kind="ExternalInput")
    t_wbT = nc.dram_tensor("wbT", [128, KT, D_OUT], F32R, kind="ExternalInput")
    t_w1 = nc.dram_tensor("w1s", [128, MP, H], F32R, kind="ExternalInput")
    t_w2 = nc.dram_tensor("w2s", [H, MP, 128], BF16, kind="ExternalInput")
    t_oma = nc.dram_tensor("omaT", [128, MP], F32, kind="ExternalInput")
    t_bb = nc.dram_tensor("bbT", [128, MP], F32, kind="ExternalInput")
    t_b1 = nc.dram_tensor("b1T", [H, S], F32, kind="ExternalInput")
    t_b2 = nc.dram_tensor("b2T", [128, MP], F32, kind="ExternalInput")
    t_out = nc.dram_tensor("outT", [D_OUT, BC], F32, kind="ExternalOutput")

    with tile.TileContext(nc) as tc:
        with (
            tc.tile_pool(name="wres", bufs=1) as wres,
            tc.tile_pool(name="wbp", bufs=3) as wbp,
            tc.tile_pool(name="yp", bufs=3) as yp,
            tc.tile_pool(name="hp", bufs=4) as hp,
            tc.tile_pool(name="op", bufs=3) as op,
            tc.tile_pool(name="psy", bufs=2, space="PSUM") as psy,
            tc.tile_pool(name="psh", bufs=4, space="PSUM") as psh,
            tc.tile_pool(name="pso", bufs=2, space="PSUM") as pso,
        ):
            def body():
                xs = wres.tile([128, KT, BC], F32R, tag="xs")
                w1 = wres.tile([128, MP, H], F32R, tag="w1")
                w2 = wres.tile([H, MP, 128], BF16, tag="w2")
                oma = wres.tile([128, MP], F32, tag="oma")
                bb = wres.tile([128, MP], F32, tag="bb")
                b1 = wres.tile([H, S], F32, tag="b1")
                b2 = wres.tile([128, MP], F32, tag="b2")
                for t, d in [(xs, t_xT), (w1, t_w1), (w2, t_w2), (oma, t_oma),
                             (bb, t_bb), (b1, t_b1), (b2, t_b2)]:
                    nc.sync.dma_start(t[:], d.ap())

                for m in range(MP):
                    wb_m = wbp.tile([128, KT, 128], F32R)
                    nc.sync.dma_start(
                        wb_m[:], t_wbT.ap()[:, :, 128 * m:128 * (m + 1)])
                    for n in range(NN):
                        nsl = slice(NT * n, NT * (n + 1))
                        ps_y = psy.tile([128, NT], F32)
                        for k in range(KT):
                            nc.tensor.matmul(
                                ps_y[:], wb_m[:, k, :], xs[:, k, nsl],
                                start=(k == 0), stop=(k == KT - 1))
                        yT = yp.tile([128, NT], F32R)
                        nc.vector.tensor_scalar_add(yT[:], ps_y[:], bb[:, m:m + 1])

                        hs = []
                        for j in range(2):
                            psl = slice(64 * j, 64 * (j + 1))
                            ps_h = psh.tile([128, NT], F32)
                            nc.tensor.matmul(
                                ps_h[:], w1[psl, m, :], yT[psl, :],
                                start=True, stop=True, tile_position=(64 * j, 0))
                            h_j = hp.tile([128, NT], BF16)
                            nc.scalar.activation(
                                h_j[:], ps_h[:],
                                mybir.ActivationFunctionType.Relu,
                                bias=b1[:, 2 * m + j:2 * m + j + 1])
                            hs.append(h_j)

                        ps_o = pso.tile([128, NT], F32)
                        nc.tensor.matmul(ps_o[0:64, :], w2[:, m, 0:64], hs[0][:],
                                         start=True, stop=True)
                        nc.tensor.matmul(ps_o[64:128, :], w2[:, m, 64:128], hs[1][:],
                                         start=True, stop=True)
                        # blend term (1-a)*y + b2' on ACT, then += bp on DVE
                        bl = op.tile([128, NT], F32, tag="bl")
                        nc.scalar.activation(
                            bl[:], yT[:],
                            mybir.ActivationFunctionType.Identity,
                            bias=b2[:, m:m + 1], scale=oma[:, m:m + 1])
                        out_t = op.tile([128, NT], F32)
                        nc.vector.tensor_tensor(out=out_t[:], in0=bl[:],
                                                in1=ps_o[:],
                                                op=mybir.AluOpType.add)
                        nc.sync.dma_start(
                            t_out.ap()[128 * m:128 * (m + 1), nsl], out_t[:])

            if hw_loop and reps > 1:
                with tc.For_i(0, reps, 1):
                    body()
            else:
                for _ in range(reps):
                    body()

    nc.compile()
    return nc


def prep_shared(Wb, bb, W1, b1, W2, b2, alpha, active):
    """Host-side packing of the replicated (per-core-identical) inputs."""
    ae = (alpha.astype(np.float32) * active.astype(np.float32))  # [S]

    wbT = np.ascontiguousarray(
        Wb.T.reshape(KT, 128, D_OUT).transpose(1, 0, 2)).astype(np.float32)
    w1s = np.ascontiguousarray(
        W1.reshape(MP, 128, H).transpose(1, 0, 2)).astype(np.float32)

    W2p = (ae[:, None, None] * W2).astype(np.float32)            # [S, H, C]
    w2s = np.ascontiguousarray(
        W2p.reshape(MP, 2, H, C).transpose(2, 0, 1, 3).reshape(H, MP, 128)
    ).astype(ml_dtypes.bfloat16)

    omaT = np.ascontiguousarray(
        np.repeat(1.0 - ae, C).astype(np.float32).reshape(MP, 128).T)

    bbT = np.ascontiguousarray(bb.reshape(MP, 128).T).astype(np.float32)
    b1T = np.ascontiguousarray(b1.T).astype(np.float32)          # [H, S]
    b2p = (ae[:, None] * b2).astype(np.float32)                  # [S, C]
    b2T = np.ascontiguousarray(b2p.reshape(MP, 128).T)

    return {"wbT": wbT, "w1s": w1s, "w2s": w2s, "omaT": omaT,
            "bbT": bbT, "b1T": b1T, "b2T": b2T}


def prep_core(x_shard):
    """x_shard [BC, D_IN] -> xT [128, KT, BC]."""
    return np.ascontiguousarray(
        x_shard.T.reshape(KT, 128, BC).transpose(1, 0, 2)).astype(np.float32)


def run(nc, in_maps):
    import time
    last = None
    for attempt in range(4):
        try:
            return bass_utils.run_bass_kernel_spmd(
                nc, in_maps, core_ids=list(range(NCORES)))
        except Exception as e:  # transient NRT_EXEC_UNIT_UNRECOVERABLE etc.
            last = e
            time.sleep(20 * (attempt + 1))
    raise last


def kernel(x, Wb, bb, W1, b1, W2, b2, alpha, active):
    nc = build(1)
    shared = prep_shared(Wb, bb, W1, b1, W2, b2, alpha, active)
    in_maps = [
        {**shared, "xT": prep_core(x[i * BC:(i + 1) * BC])}
        for i in range(NCORES)
    ]
    res = run(nc, in_maps)
    out = np.empty((B, D_OUT), np.float32)
    for i in range(NCORES):
        out[i * BC:(i + 1) * BC] = res.results[i]["outT"].T
    return out
